# revision 31
# baseline (speedup 1.0000x reference)
"""Trainium2 Bass kernel for nn_Attn (Bahdanau-style attention scores).

Reference computation:
    energy[s,b,:] = W @ enc[s,b,:] + bias          [S,B,H]
    scores[b,s]   = hidden[0,b,:] . energy[s,b,:]  [B,S]
    out           = softmax(scores, axis=-1)[:,None,:]

Key rewrite: scores[b,s] = (W^T hidden_b) . enc[s,b,:] + hidden_b . bias.
The second term is constant in s, so it is invariant under softmax and is
dropped entirely.  v_b = W^T hidden_b is a tiny [B, 2H] matvec done on the
tensor engine; the S*B*2H dot-product sweep is done by the vector engine
(elementwise multiply) + scalar engine (activation-Copy with accum_out for
the free-dim sum) while DMA streams enc at full HBM bandwidth.

Sharding: data-parallel over batch B (4 batch rows per core, 8 cores).
Each core receives enc[:, b0:b0+4, :] (64 MiB), hidden^T slice and W.
"""

import numpy as np

# Problem sizes (hardcoded per harness contract).
H = 1024          # hidden size
K = 2 * H         # 2H = contraction dim of W
S = 2048          # encoder sequence length
B = 32            # batch
N_CORES = 8
BPC = B // N_CORES  # batch rows per core = 4

ST = 128          # s-tile (partition dim)
NST = S // ST     # 16 s-tiles
KC = 512          # psum free chunk for the v matmul
NKC = K // KC     # 4
HC = 128          # h chunk (matmul contraction tile)
NHC = H // HC     # 8
BGRP = 2          # batch rows per enc DMA tile

# debug toggles (bisect)
USE_GPSIMD_RING = False  # enc DMAs also on SWDGE ring (slower: Q7 chokes)
USE_NEG_REDUCE = True    # tensor_reduce(negate=True)
USE_PE_TAIL = True       # transposed-softmax tail (vs per-partition path)

_CACHE = {}


def _emit(ctx, tc, enc, hidT, w, out):
    """Emit the per-core program.

    enc : DRAM [S, BPC, K]  fp32
    hidT: DRAM [128, NHC*BPC] fp32, layout [p][c][b] for h = c*128 + p
    w   : DRAM [H, K] fp32
    out : DRAM [BPC, S] fp32  (softmax probabilities)
    """
    from concourse import mybir
    from concourse.masks import make_identity

    nc = tc.nc
    f32 = mybir.dt.float32

    singles = ctx.enter_context(tc.tile_pool(name="singles", bufs=1))
    wpool = ctx.enter_context(tc.tile_pool(name="wpool", bufs=2))
    encpool = ctx.enter_context(tc.tile_pool(name="encp", bufs=4))
    prodpool = ctx.enter_context(tc.tile_pool(name="prodp", bufs=3))
    vpsum = ctx.enter_context(tc.tile_pool(name="vpsum", bufs=1, space="PSUM"))
    bcpsum = ctx.enter_context(tc.tile_pool(name="bcpsum", bufs=2, space="PSUM"))
    tpsum = ctx.enter_context(tc.tile_pool(name="tpsum", bufs=1, space="PSUM"))
    small = ctx.enter_context(tc.tile_pool(name="small", bufs=2))

    # ---- constants (no input deps; scheduled early) ---------------------
    ident = singles.tile([128, 128], f32)
    make_identity(nc, ident)
    ones = singles.tile([1, 128], f32)
    nc.vector.memset(ones, 1.0)

    # ---- load hidden^T (tiny) -------------------------------------------
    hid_sb = singles.tile([128, NHC * BPC], f32)
    nc.scalar.dma_start(out=hid_sb, in_=hidT)

    # ---- v = W^T h, quarter-by-quarter over k, fused with broadcast -----
    # W streams as 4 column-quarter tiles [128, NHC, KC]; quarter q's
    # matvec + partition-0 flatten + ones-matmul broadcast overlap the DMA
    # of quarter q+1, so v_bc completes right after the last W byte lands.
    v_bc = singles.tile([128, BPC, K], f32)
    v_sb = singles.tile([BPC, K], f32)
    w_dmas = []
    for q in range(NKC):
        w_sb = wpool.tile([128, NHC, KC], f32, name="w_sb", tag="w_sb")
        weng = nc.scalar if (q % 2 == 0) else nc.sync
        w_dmas.append(
            weng.dma_start(
                out=w_sb,
                in_=w[:, q * KC:(q + 1) * KC].rearrange("(c p) k -> p c k", p=HC),
            )
        )
        v_ps = vpsum.tile([BPC, KC], f32, name="v_ps", tag="v_ps", bufs=2)
        for c in range(NHC):
            nc.tensor.matmul(
                v_ps[:, :],
                lhsT=hid_sb[:, c * BPC:(c + 1) * BPC],
                rhs=w_sb[:, c, :],
                start=(c == 0),
                stop=(c == NHC - 1),
            )
        nc.scalar.copy(out=v_sb[:, q * KC:(q + 1) * KC], in_=v_ps[:, :])
        # flatten the 4 v rows of this quarter onto partition 0
        v_row = singles.tile([1, BPC * KC], f32, name="v_row", tag="v_row")
        nc.gpsimd.dma_start(out=v_row, in_=v_sb[:, q * KC:(q + 1) * KC])
        for b in range(BPC):
            bc_ps = bcpsum.tile([128, KC], f32, name="bc_ps", tag="bc_ps")
            nc.tensor.matmul(
                bc_ps[:, :],
                lhsT=ones,
                rhs=v_row[0:1, b * KC:(b + 1) * KC],
                start=True,
                stop=True,
            )
            eng = nc.vector if (q * BPC + b) % 2 == 0 else nc.scalar
            if eng is nc.vector:
                eng.tensor_copy(v_bc[:, b, q * KC:(q + 1) * KC], bc_ps[:, :])
            else:
                eng.copy(out=v_bc[:, b, q * KC:(q + 1) * KC], in_=bc_ps[:, :])

    # ---- main sweep: scores[s,b] = enc[s,b,:] . v_b ---------------------
    # DVE does the elementwise multiply; ScalarE (activation Copy with
    # accum_out) does the free-dim sum, so the two passes run on separate
    # engines and both stay under the DMA streaming time.
    scores = singles.tile([128, BPC, NST], f32)
    NBG = BPC // BGRP
    # All bulk enc DMAs issue from the sync engine: its sequencer does
    # nothing else, so descriptor generation is never delayed behind
    # compute (scalar's sequencer is saturated by the reduce chain).
    enc_rings = [nc.sync]
    from concourse.bass import _add_dep_helper

    for st in range(NST):
        for g in range(NBG):
            enc_sb = encpool.tile([128, BGRP, K], f32)
            eng = enc_rings[(st * NBG + g) % len(enc_rings)]
            enc_dma = eng.dma_start(
                out=enc_sb,
                in_=enc[st * ST:(st + 1) * ST, g * BGRP:(g + 1) * BGRP, :],
            )
            i = st * NBG + g
            if i < 4:
                # the W phase is DMA-bound (~19us at full rate): hold the
                # first enc DMAs until every W quarter has landed so enc
                # traffic never delays the v chain on the rings
                _add_dep_helper(
                    enc_dma.ins,
                    w_dmas[-1].ins,
                    reason="enc stream yields to W prologue",
                )
            for bi in range(BGRP):
                b = g * BGRP + bi
                prod = prodpool.tile([128, K], f32, name="prod", tag="prod")
                nc.vector.tensor_mul(prod, enc_sb[:, bi, :], v_bc[:, b, :])
                nc.scalar.activation(
                    out=prod,
                    in_=prod,
                    func=mybir.ActivationFunctionType.Copy,
                    bias=0.0,
                    scale=1.0,
                    accum_out=scores[:, b, st:st + 1],
                )

    # ---- softmax over s, in transposed [BPC, S] layout ------------------
    # scores [128 s_in, (b t)] -> PE transpose -> [(b t), s_in] -> SBUF->SBUF
    # DMA reshape -> s4 [BPC, S].  Then softmax is a single free-axis chain:
    # -max (negated reduce), in-place exp with bias + fused denominator
    # accum, reciprocal, in-place scale, natural-layout store.
    sc2 = scores.rearrange("p b t -> p (b t)")
    scT_ps = tpsum.tile([BPC * NST, 128], f32)
    nc.tensor.transpose(scT_ps[:, :], sc2, ident[:, :])
    scT = small.tile([BPC * NST, 128], f32)
    nc.vector.tensor_copy(scT, scT_ps[:, :])
    s4 = singles.tile([BPC, S], f32)
    nc.sync.dma_start(out=s4, in_=scT)

    nm4 = small.tile([BPC, 1], f32)
    if USE_NEG_REDUCE:
        nc.vector.tensor_reduce(
            out=nm4, in_=s4, axis=mybir.AxisListType.X, op=mybir.AluOpType.max,
            negate=True,
        )
    else:
        m4 = small.tile([BPC, 1], f32)
        nc.vector.tensor_reduce(
            out=m4, in_=s4, axis=mybir.AxisListType.X, op=mybir.AluOpType.max
        )
        nc.vector.tensor_scalar_mul(nm4, m4, -1.0)
    r4 = small.tile([BPC, 1], f32)
    nc.scalar.activation(
        out=s4,
        in_=s4,
        func=mybir.ActivationFunctionType.Exp,
        bias=nm4,
        scale=1.0,
        accum_out=r4,
    )
    inv4 = small.tile([BPC, 1], f32)
    nc.vector.reciprocal(inv4, r4)
    nc.vector.tensor_scalar_mul(s4, s4, inv4)
    nc.sync.dma_start(out=out, in_=s4)


def _build():
    if "nc" in _CACHE:
        return _CACHE["nc"]
    from contextlib import ExitStack

    import concourse.bacc as bacc
    import concourse.tile as tile
    from concourse import mybir

    nc = bacc.Bacc(
        "TRN2", target_bir_lowering=False, debug=False, num_devices=N_CORES
    )
    enc_d = nc.dram_tensor("enc", [S, BPC, K], mybir.dt.float32, kind="ExternalInput")
    hid_d = nc.dram_tensor(
        "hidT", [128, NHC * BPC], mybir.dt.float32, kind="ExternalInput"
    )
    w_d = nc.dram_tensor("w", [H, K], mybir.dt.float32, kind="ExternalInput")
    out_d = nc.dram_tensor(
        "attn_out", [BPC, S], mybir.dt.float32, kind="ExternalOutput"
    )

    with tile.TileContext(nc) as tc:
        with ExitStack() as ctx:
            _emit(ctx, tc, enc_d.ap(), hid_d.ap(), w_d.ap(), out_d.ap())
    nc.compile()
    _CACHE["nc"] = nc
    return nc


def _make_in_maps(hidden, encoder_outputs, W):
    in_maps = []
    w = np.ascontiguousarray(W, dtype=np.float32)
    for i in range(N_CORES):
        b0 = i * BPC
        # hidT layout [p][c][b] with h = c*128 + p
        hid = hidden[0, b0:b0 + BPC, :]                    # [BPC, H]
        hidT = np.ascontiguousarray(
            hid.T.reshape(NHC, 128, BPC).transpose(1, 0, 2).reshape(128, NHC * BPC),
            dtype=np.float32,
        )
        enc = np.ascontiguousarray(
            encoder_outputs[:, b0:b0 + BPC, :], dtype=np.float32
        )
        in_maps.append({"enc": enc, "hidT": hidT, "w": w})
    return in_maps


def kernel(hidden, encoder_outputs, W, b):
    from concourse import bass_utils

    nc = _build()
    in_maps = _make_in_maps(
        np.asarray(hidden), np.asarray(encoder_outputs), np.asarray(W)
    )
    res = bass_utils.run_bass_kernel_spmd(
        nc, in_maps, core_ids=list(range(N_CORES))
    )
    out = np.concatenate(
        [res.results[i]["attn_out"] for i in range(N_CORES)], axis=0
    )  # [B, S]
    return out[:, None, :].astype(np.float32)


# revision 34
# speedup vs baseline: 1.0633x; 1.0633x over previous
"""Trainium2 Bass kernel for nn_Attn (Bahdanau-style attention scores).

Reference computation:
    energy[s,b,:] = W @ enc[s,b,:] + bias          [S,B,H]
    scores[b,s]   = hidden[0,b,:] . energy[s,b,:]  [B,S]
    out           = softmax(scores, axis=-1)[:,None,:]

Key rewrite: scores[b,s] = (W^T hidden_b) . enc[s,b,:] + hidden_b . bias.
The second term is constant in s, so it is invariant under softmax and is
dropped entirely.  v_b = W^T hidden_b is a tiny [B, 2H] matvec done on the
tensor engine; the S*B*2H dot-product sweep is done by the vector engine
(elementwise multiply) + scalar engine (activation-Copy with accum_out for
the free-dim sum) while DMA streams enc at full HBM bandwidth.

Sharding: data-parallel over batch B (4 batch rows per core, 8 cores).
Each core receives enc[:, b0:b0+4, :] (64 MiB), hidden^T slice and W.
"""

import numpy as np

# Problem sizes (hardcoded per harness contract).
H = 1024          # hidden size
K = 2 * H         # 2H = contraction dim of W
S = 2048          # encoder sequence length
B = 32            # batch
N_CORES = 8
BPC = B // N_CORES  # batch rows per core = 4

ST = 128          # s-tile (partition dim)
NST = S // ST     # 16 s-tiles
KC = 512          # psum free chunk for the v matmul
NKC = K // KC     # 4
HC = 128          # h chunk (matmul contraction tile)
NHC = H // HC     # 8
BGRP = 2          # batch rows per enc DMA tile

# debug toggles (bisect)
USE_GPSIMD_RING = False  # enc DMAs also on SWDGE ring (slower: Q7 chokes)
USE_NEG_REDUCE = True    # tensor_reduce(negate=True)
USE_PE_TAIL = True       # transposed-softmax tail (vs per-partition path)

_CACHE = {}


def _emit(ctx, tc, enc, hidT, w, out):
    """Emit the per-core program.

    enc : DRAM [S, BPC, K]  fp32
    hidT: DRAM [128, NHC*BPC] fp32, layout [p][c][b] for h = c*128 + p
    w   : DRAM [H, K] fp32
    out : DRAM [BPC, S] fp32  (softmax probabilities)
    """
    from concourse import mybir
    from concourse.masks import make_identity

    nc = tc.nc
    f32 = mybir.dt.float32

    singles = ctx.enter_context(tc.tile_pool(name="singles", bufs=1))
    wpool = ctx.enter_context(tc.tile_pool(name="wpool", bufs=2))
    encpool = ctx.enter_context(tc.tile_pool(name="encp", bufs=4))
    prodpool = ctx.enter_context(tc.tile_pool(name="prodp", bufs=3))
    vpsum = ctx.enter_context(tc.tile_pool(name="vpsum", bufs=1, space="PSUM"))
    bcpsum = ctx.enter_context(tc.tile_pool(name="bcpsum", bufs=2, space="PSUM"))
    tpsum = ctx.enter_context(tc.tile_pool(name="tpsum", bufs=1, space="PSUM"))
    small = ctx.enter_context(tc.tile_pool(name="small", bufs=2))

    # ---- constants (no input deps; scheduled early) ---------------------
    ident = singles.tile([128, 128], f32)
    make_identity(nc, ident)
    ones = singles.tile([1, 128], f32)
    nc.vector.memset(ones, 1.0)

    # ---- PE warm-up ------------------------------------------------------
    # TensorE clocks at 1.2 GHz until it has been busy ~4us, then 2.4 GHz.
    # The v chain is PE-bound, so burn dummy matmuls on a scratch PSUM bank
    # while the W DMAs stream: the real matmuls then run at full clock.
    warm_ps = bcpsum.tile([128, 128], f32, name="warm_ps", tag="warm_ps")
    for _ in range(36):
        nc.tensor.matmul(
            warm_ps[:, :], lhsT=ident, rhs=ident, start=True, stop=True
        )

    # ---- load hidden^T (tiny) -------------------------------------------
    hid_sb = singles.tile([128, NHC * BPC], f32)
    nc.scalar.dma_start(out=hid_sb, in_=hidT)

    # ---- v = W^T h, quarter-by-quarter over k, fused with broadcast -----
    # W streams as 4 column-quarter tiles [128, NHC, KC]; quarter q's
    # matvec + partition-0 flatten + ones-matmul broadcast overlap the DMA
    # of quarter q+1, so v_bc completes right after the last W byte lands.
    v_bc = singles.tile([128, BPC, K], f32)
    v_sb = singles.tile([BPC, K], f32)
    w_dmas = []
    for q in range(NKC):
        w_sb = wpool.tile([128, NHC, KC], f32, name="w_sb", tag="w_sb")
        weng = nc.scalar if (q % 2 == 0) else nc.sync
        w_dmas.append(
            weng.dma_start(
                out=w_sb,
                in_=w[:, q * KC:(q + 1) * KC].rearrange("(c p) k -> p c k", p=HC),
            )
        )
        v_ps = vpsum.tile([BPC, KC], f32, name="v_ps", tag="v_ps", bufs=2)
        for c in range(NHC):
            nc.tensor.matmul(
                v_ps[:, :],
                lhsT=hid_sb[:, c * BPC:(c + 1) * BPC],
                rhs=w_sb[:, c, :],
                start=(c == 0),
                stop=(c == NHC - 1),
            )
        nc.scalar.copy(out=v_sb[:, q * KC:(q + 1) * KC], in_=v_ps[:, :])
        # flatten the 4 v rows of this quarter onto partition 0
        v_row = singles.tile([1, BPC * KC], f32, name="v_row", tag="v_row")
        nc.gpsimd.dma_start(out=v_row, in_=v_sb[:, q * KC:(q + 1) * KC])
        for b in range(BPC):
            bc_ps = bcpsum.tile([128, KC], f32, name="bc_ps", tag="bc_ps")
            nc.tensor.matmul(
                bc_ps[:, :],
                lhsT=ones,
                rhs=v_row[0:1, b * KC:(b + 1) * KC],
                start=True,
                stop=True,
            )
            eng = nc.vector if (q * BPC + b) % 2 == 0 else nc.scalar
            if eng is nc.vector:
                eng.tensor_copy(v_bc[:, b, q * KC:(q + 1) * KC], bc_ps[:, :])
            else:
                eng.copy(out=v_bc[:, b, q * KC:(q + 1) * KC], in_=bc_ps[:, :])

    # ---- main sweep: scores[s,b] = enc[s,b,:] . v_b ---------------------
    # DVE does the elementwise multiply; ScalarE (activation Copy with
    # accum_out) does the free-dim sum, so the two passes run on separate
    # engines and both stay under the DMA streaming time.
    scores = singles.tile([128, BPC, NST], f32)
    NBG = BPC // BGRP
    # All bulk enc DMAs issue from the sync engine: its sequencer does
    # nothing else, so descriptor generation is never delayed behind
    # compute (scalar's sequencer is saturated by the reduce chain).
    enc_rings = [nc.sync]
    from concourse.bass import _add_dep_helper

    for st in range(NST):
        for g in range(NBG):
            enc_sb = encpool.tile([128, BGRP, K], f32)
            eng = enc_rings[(st * NBG + g) % len(enc_rings)]
            enc_dma = eng.dma_start(
                out=enc_sb,
                in_=enc[st * ST:(st + 1) * ST, g * BGRP:(g + 1) * BGRP, :],
            )
            i = st * NBG + g
            if i < 4:
                # the W phase is DMA-bound (~19us at full rate): hold the
                # first enc DMAs until every W quarter has landed so enc
                # traffic never delays the v chain on the rings
                _add_dep_helper(
                    enc_dma.ins,
                    w_dmas[-1].ins,
                    reason="enc stream yields to W prologue",
                )
            for bi in range(BGRP):
                b = g * BGRP + bi
                prod = prodpool.tile([128, K], f32, name="prod", tag="prod")
                nc.vector.tensor_mul(prod, enc_sb[:, bi, :], v_bc[:, b, :])
                nc.scalar.activation(
                    out=prod,
                    in_=prod,
                    func=mybir.ActivationFunctionType.Copy,
                    bias=0.0,
                    scale=1.0,
                    accum_out=scores[:, b, st:st + 1],
                )

    # ---- softmax over s, in transposed [BPC, S] layout ------------------
    # scores [128 s_in, (b t)] -> PE transpose -> [(b t), s_in] -> SBUF->SBUF
    # DMA reshape -> s4 [BPC, S].  Then softmax is a single free-axis chain:
    # -max (negated reduce), in-place exp with bias + fused denominator
    # accum, reciprocal, in-place scale, natural-layout store.
    sc2 = scores.rearrange("p b t -> p (b t)")
    scT_ps = tpsum.tile([BPC * NST, 128], f32)
    nc.tensor.transpose(scT_ps[:, :], sc2, ident[:, :])
    scT = small.tile([BPC * NST, 128], f32)
    nc.vector.tensor_copy(scT, scT_ps[:, :])
    s4 = singles.tile([BPC, S], f32)
    nc.sync.dma_start(out=s4, in_=scT)

    nm4 = small.tile([BPC, 1], f32)
    if USE_NEG_REDUCE:
        nc.vector.tensor_reduce(
            out=nm4, in_=s4, axis=mybir.AxisListType.X, op=mybir.AluOpType.max,
            negate=True,
        )
    else:
        m4 = small.tile([BPC, 1], f32)
        nc.vector.tensor_reduce(
            out=m4, in_=s4, axis=mybir.AxisListType.X, op=mybir.AluOpType.max
        )
        nc.vector.tensor_scalar_mul(nm4, m4, -1.0)
    r4 = small.tile([BPC, 1], f32)
    nc.scalar.activation(
        out=s4,
        in_=s4,
        func=mybir.ActivationFunctionType.Exp,
        bias=nm4,
        scale=1.0,
        accum_out=r4,
    )
    inv4 = small.tile([BPC, 1], f32)
    nc.vector.reciprocal(inv4, r4)
    nc.vector.tensor_scalar_mul(s4, s4, inv4)
    nc.sync.dma_start(out=out, in_=s4)


def _build():
    if "nc" in _CACHE:
        return _CACHE["nc"]
    from contextlib import ExitStack

    import concourse.bacc as bacc
    import concourse.tile as tile
    from concourse import mybir

    nc = bacc.Bacc(
        "TRN2", target_bir_lowering=False, debug=False, num_devices=N_CORES
    )
    enc_d = nc.dram_tensor("enc", [S, BPC, K], mybir.dt.float32, kind="ExternalInput")
    hid_d = nc.dram_tensor(
        "hidT", [128, NHC * BPC], mybir.dt.float32, kind="ExternalInput"
    )
    w_d = nc.dram_tensor("w", [H, K], mybir.dt.float32, kind="ExternalInput")
    out_d = nc.dram_tensor(
        "attn_out", [BPC, S], mybir.dt.float32, kind="ExternalOutput"
    )

    with tile.TileContext(nc) as tc:
        with ExitStack() as ctx:
            _emit(ctx, tc, enc_d.ap(), hid_d.ap(), w_d.ap(), out_d.ap())
    nc.compile()
    _CACHE["nc"] = nc
    return nc


def _make_in_maps(hidden, encoder_outputs, W):
    in_maps = []
    w = np.ascontiguousarray(W, dtype=np.float32)
    for i in range(N_CORES):
        b0 = i * BPC
        # hidT layout [p][c][b] with h = c*128 + p
        hid = hidden[0, b0:b0 + BPC, :]                    # [BPC, H]
        hidT = np.ascontiguousarray(
            hid.T.reshape(NHC, 128, BPC).transpose(1, 0, 2).reshape(128, NHC * BPC),
            dtype=np.float32,
        )
        enc = np.ascontiguousarray(
            encoder_outputs[:, b0:b0 + BPC, :], dtype=np.float32
        )
        in_maps.append({"enc": enc, "hidT": hidT, "w": w})
    return in_maps


def kernel(hidden, encoder_outputs, W, b):
    from concourse import bass_utils

    nc = _build()
    in_maps = _make_in_maps(
        np.asarray(hidden), np.asarray(encoder_outputs), np.asarray(W)
    )
    res = bass_utils.run_bass_kernel_spmd(
        nc, in_maps, core_ids=list(range(N_CORES))
    )
    out = np.concatenate(
        [res.results[i]["attn_out"] for i in range(N_CORES)], axis=0
    )  # [B, S]
    return out[:, None, :].astype(np.float32)
